# revision 16
# baseline (speedup 1.0000x reference)
"""CRF NLL loss kernel for Trainium2 (8 NeuronCores, batch-parallel).

Strategy: shard the 4096-sentence batch across 8 cores (512 each). Per core,
run the CRF forward recursion in probability space with tags on partitions:
126 partitions = 14 groups x 9 body-tags; block-diagonal exp(transitions) as
stationary PE weights; each time step is one matmul (PE) + one elementwise
multiply by exp(feats) (DVE). The gold path score is computed by a parallel
"beta" recursion (one-hot-masked emission factors selects exactly the gold
path term of the forward sum), so forward and gold share the same per-step
instructions on a 74-wide free axis (37 alpha sentences | 37 beta sentences
per group). Log-scale carry C is maintained by rescaling every 128 steps.
"""
import os
import sys

import numpy as np

sys.path.insert(0, "/opt/trn_rl_repo")

from contextlib import ExitStack

import concourse.bacc as bacc
import concourse.bass as bass
import concourse.tile as tile
from concourse import mybir
from concourse.bass_utils import run_bass_kernel_spmd

# problem constants (hardcoded per spec)
B, T, K = 4096, 2048, 11
START, STOP = 10, 9
NCORES = 8
BL = B // NCORES          # 512 sentences per core
G, KT, J = 14, 9, 37      # groups x body-tags x sentences-per-group (518 slots)
P = 128                   # padded partitions (126 live = G*KT, 2 dead)
PL = G * KT               # live partitions
W = 2 * J                 # 74 free: [alpha | beta]
TC = 128                  # chunk length
NCHUNK = T // TC
RS = 32                   # rescale cadence (steps)
LNSCALE = 2.0 ** -16      # pre-scale for ACT Ln (valid range is +-2^64);
                          # the ln(2^-32) offsets cancel between halves
C0A, C0B = 3.2, 0.5       # per-step log recentering for alpha / beta chains

F32 = mybir.dt.float32
BF16 = mybir.dt.bfloat16
I8 = mybir.dt.int8


def _build_nc():
    nc = bacc.Bacc()
    f_in = nc.declare_dram_parameter("feats_t", [P, T, J], F32, isOutput=False)
    g_in = nc.declare_dram_parameter("tags_t", [P, T, J], I8, isOutput=False)
    bd_in = nc.declare_dram_parameter("bd_lhst", [P, P], BF16, isOutput=False)
    astart_in = nc.declare_dram_parameter("astart", [P, 1], F32, isOutput=False)
    astop_in = nc.declare_dram_parameter("astop", [P, G], BF16, isOutput=False)
    ones_in = nc.declare_dram_parameter("ones_bd", [P, G], BF16, isOutput=False)
    bcast_in = nc.declare_dram_parameter("bcast", [G, P], F32, isOutput=False)
    kcol_in = nc.declare_dram_parameter("kcol", [P, 1], I8, isOutput=False)
    out_ext = nc.declare_dram_parameter("nll", [G, J], F32, isOutput=True)

    with tile.TileContext(nc) as tc, ExitStack() as ctx:
        consts = ctx.enter_context(tc.tile_pool(name="consts", bufs=1))
        feats_pool = ctx.enter_context(tc.tile_pool(name="feats", bufs=2))
        tags_pool = ctx.enter_context(tc.tile_pool(name="tags", bufs=2))
        e_pool = ctx.enter_context(tc.tile_pool(name="ecomb", bufs=2))
        state_pool = ctx.enter_context(tc.tile_pool(name="state", bufs=3))
        small_pool = ctx.enter_context(tc.tile_pool(name="small", bufs=2))
        psum_pool = ctx.enter_context(
            tc.tile_pool(name="psum", bufs=4, space="PSUM"))

        bd = consts.tile([P, P], BF16)
        nc.sync.dma_start(out=bd, in_=bd_in[:])
        astart = consts.tile([P, 1], F32)
        nc.sync.dma_start(out=astart, in_=astart_in[:])
        astop = consts.tile([P, G], BF16)
        nc.sync.dma_start(out=astop, in_=astop_in[:])
        ones_bd = consts.tile([P, G], BF16)
        nc.sync.dma_start(out=ones_bd, in_=ones_in[:])
        bcast = consts.tile([G, P], F32)
        nc.sync.dma_start(out=bcast, in_=bcast_in[:])
        kcol = consts.tile([P, 1], I8)
        nc.sync.dma_start(out=kcol, in_=kcol_in[:])

        cacc = consts.tile([G, W], F32)
        nc.vector.memset(cacc, 0.0)
        bias_a = consts.tile([P, 1], F32)
        nc.vector.memset(bias_a, -C0A)
        bias_b = consts.tile([P, 1], F32)
        nc.vector.memset(bias_b, -C0B)
        lnscale = consts.tile([G, 1], F32)
        nc.vector.memset(lnscale, LNSCALE)

        alpha = None
        for chunk in range(NCHUNK):
            ft = feats_pool.tile([P, TC, J], F32, tag="ft")
            nc.sync.dma_start(out=ft, in_=f_in[:, chunk * TC:(chunk + 1) * TC, :])
            tg = tags_pool.tile([P, TC, J], I8, tag="tg")
            nc.sync.dma_start(out=tg, in_=g_in[:, chunk * TC:(chunk + 1) * TC, :])
            ec = e_pool.tile([P, TC, W], F32, tag="ec")
            nc.scalar.activation(
                out=ec[:, :, 0:J], in_=ft,
                func=mybir.ActivationFunctionType.Exp, bias=bias_a, scale=1.0)
            nc.scalar.activation(
                out=ec[:, :, J:W], in_=ft,
                func=mybir.ActivationFunctionType.Exp, bias=bias_b, scale=1.0)
            # beta half: keep only the gold-tag emission factor
            nc.vector.scalar_tensor_tensor(
                out=ec[:, :, J:W], in0=tg, scalar=kcol, in1=ec[:, :, J:W],
                op0=mybir.AluOpType.is_equal, op1=mybir.AluOpType.mult)

            for t in range(TC):
                tau = chunk * TC + t
                if tau == 0:
                    alpha = state_pool.tile([P, W], BF16, tag="alpha")
                    nc.vector.tensor_scalar_mul(
                        out=alpha, in0=ec[:, 0, :], scalar1=astart)
                else:
                    ps = psum_pool.tile([P, W], F32, tag="ps")
                    nc.tensor.matmul(ps, bd, alpha, start=True, stop=True)
                    alpha = state_pool.tile([P, W], BF16, tag="alpha")
                    nc.vector.tensor_mul(out=alpha, in0=ps, in1=ec[:, t, :])

                if (tau + 1) % RS == 0 and (tau + 1) < T:
                    s_ps = psum_pool.tile([G, W], F32, tag="sps")
                    nc.tensor.matmul(s_ps, ones_bd, alpha, start=True, stop=True)
                    r_sb = small_pool.tile([G, W], F32, tag="r")
                    nc.vector.reciprocal(out=r_sb, in_=s_ps)
                    ln_sb = small_pool.tile([G, W], F32, tag="ln")
                    nc.scalar.activation(
                        out=ln_sb, in_=s_ps,
                        func=mybir.ActivationFunctionType.Ln, scale=lnscale)
                    nc.vector.tensor_add(out=cacc, in0=cacc, in1=ln_sb)
                    rb_ps = psum_pool.tile([P, W], F32, tag="ps")
                    nc.tensor.matmul(rb_ps, bcast, r_sb, start=True, stop=True)
                    rb_sb = state_pool.tile([P, W], BF16, tag="rb")
                    nc.scalar.activation(
                        out=rb_sb, in_=rb_ps,
                        func=mybir.ActivationFunctionType.Copy)
                    alpha_new = state_pool.tile([P, W], BF16, tag="alpha")
                    nc.vector.tensor_mul(out=alpha_new, in0=alpha, in1=rb_sb)
                    alpha = alpha_new

        f_ps = psum_pool.tile([G, W], F32, tag="sps")
        nc.tensor.matmul(f_ps, astop, alpha, start=True, stop=True)
        ln_f = small_pool.tile([G, W], F32, tag="ln")
        nc.scalar.activation(
            out=ln_f, in_=f_ps, func=mybir.ActivationFunctionType.Ln,
            scale=lnscale)
        nc.vector.tensor_add(out=cacc, in0=cacc, in1=ln_f)

        nll_sb = small_pool.tile([G, J], F32, tag="nll")
        nc.vector.tensor_sub(out=nll_sb, in0=cacc[:, 0:J], in1=cacc[:, J:W])
        nc.vector.tensor_scalar_add(
            out=nll_sb, in0=nll_sb, scalar1=float(T) * (C0A - C0B))
        nc.sync.dma_start(out=out_ext[:], in_=nll_sb)

    nc.finalize()
    return nc


def _host_prep(feats, tags, transitions):
    """Build per-core input maps. Layout/dtype staging only — all FLOPs on device
    except the 11x11 exp(transitions) weight build."""
    import ml_dtypes
    f32 = np.float32
    bf16 = ml_dtypes.bfloat16
    feats = np.asarray(feats, dtype=f32)
    tags_i = np.asarray(tags).astype(np.int8)
    trans = np.asarray(transitions, dtype=f32)

    def padp(a):
        """pad partition (first) dim from PL=126 to P=128 with zeros"""
        out = np.zeros((P,) + a.shape[1:], dtype=a.dtype)
        out[:PL] = a
        return np.ascontiguousarray(out)

    A = np.exp(trans.astype(np.float64)).astype(f32)     # A[next, prev]
    Abody = A[:KT, :KT]
    eye = np.eye(G, dtype=f32)
    bd0 = np.kron(eye, Abody.T)                          # [126,126]
    bd = np.zeros((P, P), dtype=bf16)
    bd[:PL, :PL] = bd0.astype(bf16)
    astart = padp(np.tile(A[:KT, START], G)[:, None].astype(f32))
    astop = padp(np.kron(eye, A[STOP, :KT].reshape(KT, 1)).astype(bf16))
    ones_bd = padp(np.kron(eye, np.ones((KT, 1), f32)).astype(bf16))
    bcast = np.zeros((G, P), dtype=f32)
    bcast[:, :PL] = np.kron(eye, np.ones((1, KT), f32))
    kcol = padp(np.tile(np.arange(KT, dtype=np.int8), G)[:, None])
    kcol[PL:] = -1

    nslots = G * J
    in_maps = []
    for c in range(NCORES):
        fb = feats[c * BL:(c + 1) * BL, :, :KT]
        tb = tags_i[c * BL:(c + 1) * BL]
        fpad = np.zeros((nslots, T, KT), dtype=f32)
        fpad[:BL] = fb
        tpad = np.zeros((nslots, T), dtype=np.int8)
        tpad[:BL] = tb
        feats_T = padp(
            fpad.reshape(G, J, T, KT).transpose(0, 3, 2, 1).reshape(PL, T, J))
        tags_T = padp(
            np.ascontiguousarray(np.broadcast_to(
                tpad.reshape(G, J, T).transpose(0, 2, 1)[:, None, :, :],
                (G, KT, T, J))).reshape(PL, T, J))
        in_maps.append({
            "feats_t": feats_T,
            "tags_t": tags_T,
            "bd_lhst": bd,
            "astart": astart,
            "astop": astop,
            "ones_bd": ones_bd,
            "bcast": bcast,
            "kcol": kcol,
        })
    return in_maps


LAST_EXEC_NS = None


def kernel(feats, tags, transitions):
    global LAST_EXEC_NS
    in_maps = _host_prep(feats, tags, transitions)
    nc = _build_nc()
    trace = os.environ.get("KERNEL_TRACE") == "1"
    res = run_bass_kernel_spmd(nc, in_maps, list(range(NCORES)), trace=trace)
    LAST_EXEC_NS = res.exec_time_ns
    outs = []
    for c in range(NCORES):
        nll_parts = np.asarray(res.results[c]["nll"], dtype=np.float32)
        outs.append(nll_parts.reshape(-1)[:BL])
    return np.concatenate(outs).astype(np.float32)


if __name__ == "__main__":
    rng = np.random.default_rng(0)
    feats = rng.standard_normal((B, T, K), dtype=np.float32)
    tags = rng.integers(0, 9, size=(B, T), dtype=np.int64)
    trans = rng.random((K, K), dtype=np.float32)
    trans[START, :] = -10000.0
    trans[:, STOP] = -10000.0
    out = kernel(feats=feats, tags=tags, transitions=trans)
    print(out.shape, out[:4])


# revision 23
# speedup vs baseline: 93.3284x; 93.3284x over previous
"""CRF NLL loss kernel for Trainium2 (8 NeuronCores, batch-parallel).

Strategy: shard the 4096-sentence batch across 8 cores (512 each). Per core,
run the CRF forward recursion in probability space with tags on partitions:
126 partitions = 14 groups x 9 body-tags; block-diagonal exp(transitions) as
stationary PE weights; each time step is one matmul (PE) + one elementwise
multiply by exp(feats) (DVE). The gold path score is computed by a parallel
"beta" recursion (one-hot-masked emission factors selects exactly the gold
path term of the forward sum), so forward and gold share the same per-step
instructions on a 74-wide free axis (37 alpha sentences | 37 beta sentences
per group). A log-scale carry C is maintained by rescaling every 32 steps
(the ScalarE Ln LUT loses precision for large-magnitude inputs, so sums are
kept in a narrow range and pre-scaled by 2^-16 before Ln; the ln-offsets
cancel between the alpha and beta halves).
"""
import os
import sys

import numpy as np

sys.path.insert(0, "/opt/trn_rl_repo")

from contextlib import ExitStack

import concourse.bacc as bacc
import concourse.bass as bass
import concourse.tile as tile
from concourse import mybir
from concourse.bass_utils import run_bass_kernel_spmd

# problem constants (hardcoded per spec)
B, T, K = 4096, 2048, 11
START, STOP = 10, 9
NCORES = 8
BL = B // NCORES          # 512 sentences per core
G, KT, J = 14, 9, 37      # groups x body-tags x sentences-per-group (518 slots)
P = 128                   # padded partitions (126 live = G*KT, 2 dead)
PL = G * KT               # live partitions
W = 2 * J                 # 74 free: [alpha | beta]
TC = 128                  # chunk length
NCHUNK = T // TC
RS = 32                   # rescale cadence (steps)
LNSCALE = 2.0 ** -18      # pre-scale for ACT Ln (valid range is +-2^64);
                          # the ln(2^-32) offsets cancel between halves
C0A, C0B = 3.2, 0.5       # per-step log recentering for alpha / beta chains

F32 = mybir.dt.float32
BF16 = mybir.dt.bfloat16
I8 = mybir.dt.int8


def _build_nc(nrep=1):
    nc = bacc.Bacc()
    f_in = nc.declare_dram_parameter("feats_t", [P, T, J], F32, isOutput=False)
    g_in = nc.declare_dram_parameter("tags_t", [P, T, J], I8, isOutput=False)
    bd_in = nc.declare_dram_parameter("bd_lhst", [P, P], BF16, isOutput=False)
    astart_in = nc.declare_dram_parameter("astart", [P, 1], F32, isOutput=False)
    astop_in = nc.declare_dram_parameter("astop", [P, G], BF16, isOutput=False)
    ones_in = nc.declare_dram_parameter("ones_bd", [P, G], BF16, isOutput=False)
    bcast_in = nc.declare_dram_parameter("bcast", [G, P], F32, isOutput=False)
    kcol_in = nc.declare_dram_parameter("kcol", [P, 1], I8, isOutput=False)
    out_ext = nc.declare_dram_parameter("nll", [G, J], F32, isOutput=True)

    with tile.TileContext(nc) as tc, ExitStack() as ctx:
        consts = ctx.enter_context(tc.tile_pool(name="consts", bufs=1))
        feats_pool = ctx.enter_context(tc.tile_pool(name="feats", bufs=2))
        tags_pool = ctx.enter_context(tc.tile_pool(name="tags", bufs=2))
        e_pool = ctx.enter_context(tc.tile_pool(name="ecomb", bufs=2))
        state_pool = ctx.enter_context(tc.tile_pool(name="state", bufs=3))
        small_pool = ctx.enter_context(tc.tile_pool(name="small", bufs=2))
        psum_pool = ctx.enter_context(
            tc.tile_pool(name="psum", bufs=4, space="PSUM"))

        bd = consts.tile([P, P], BF16)
        nc.sync.dma_start(out=bd, in_=bd_in[:])
        astart = consts.tile([P, 1], F32)
        nc.sync.dma_start(out=astart, in_=astart_in[:])
        astop = consts.tile([P, G], BF16)
        nc.sync.dma_start(out=astop, in_=astop_in[:])
        ones_bd = consts.tile([P, G], BF16)
        nc.sync.dma_start(out=ones_bd, in_=ones_in[:])
        bcast = consts.tile([G, P], F32)
        nc.sync.dma_start(out=bcast, in_=bcast_in[:])
        kcol = consts.tile([P, 1], I8)
        nc.sync.dma_start(out=kcol, in_=kcol_in[:])

        cacc = consts.tile([G, W], F32)
        nc.vector.memset(cacc, 0.0)
        bias_a = consts.tile([P, 1], F32)
        nc.vector.memset(bias_a, -C0A)
        bias_b = consts.tile([P, 1], F32)
        nc.vector.memset(bias_b, -C0B)
        lnscale = consts.tile([G, 1], F32)
        nc.vector.memset(lnscale, LNSCALE)

        alpha = None
        for rep in range(nrep):
          for chunk in range(NCHUNK):
            ft = feats_pool.tile([P, TC, J], F32, tag="ft")
            nc.sync.dma_start(out=ft, in_=f_in[:, chunk * TC:(chunk + 1) * TC, :])
            tg = tags_pool.tile([P, TC, J], I8, tag="tg")
            nc.sync.dma_start(out=tg, in_=g_in[:, chunk * TC:(chunk + 1) * TC, :])
            ec = e_pool.tile([P, TC, W], F32, tag="ec")
            nc.scalar.activation(
                out=ec[:, :, 0:J], in_=ft,
                func=mybir.ActivationFunctionType.Exp, bias=bias_a, scale=1.0)
            nc.scalar.activation(
                out=ec[:, :, J:W], in_=ft,
                func=mybir.ActivationFunctionType.Exp, bias=bias_b, scale=1.0)
            # beta half: keep only the gold-tag emission factor
            nc.vector.scalar_tensor_tensor(
                out=ec[:, :, J:W], in0=tg, scalar=kcol, in1=ec[:, :, J:W],
                op0=mybir.AluOpType.is_equal, op1=mybir.AluOpType.mult)

            for t in range(TC):
                tau = chunk * TC + t
                if tau == 0:
                    alpha = state_pool.tile([P, W], BF16, tag="alpha")
                    nc.vector.tensor_scalar_mul(
                        out=alpha, in0=ec[:, 0, :], scalar1=astart)
                else:
                    ps = psum_pool.tile([P, W], F32, tag="ps")
                    nc.tensor.matmul(ps, bd, alpha, start=True, stop=True)
                    alpha = state_pool.tile([P, W], BF16, tag="alpha")
                    nc.vector.tensor_mul(out=alpha, in0=ps, in1=ec[:, t, :])

                # Rescale off the critical chain: measure S = sum_k alpha at
                # local steps {24,56,88,120}, then fold 1/S into the emission
                # slice 4 steps ahead (ec[:, t+4, :]) — the serial PE<->DVE
                # chain is never blocked, and the DVE scale-mul rides the DVE
                # program order (no extra cross-engine hops).
                if t % 32 == 24:
                    s_ps = psum_pool.tile([G, W], F32, tag="sps")
                    nc.tensor.matmul(s_ps, ones_bd, alpha, start=True, stop=True)
                    r_sb = small_pool.tile([G, W], F32, tag="r")
                    nc.vector.reciprocal(out=r_sb, in_=s_ps)
                    ln_sb = small_pool.tile([G, W], F32, tag="ln")
                    nc.scalar.activation(
                        out=ln_sb, in_=s_ps,
                        func=mybir.ActivationFunctionType.Ln, scale=lnscale)
                    nc.vector.tensor_add(out=cacc, in0=cacc, in1=ln_sb)
                    rb_ps = psum_pool.tile([P, W], F32, tag="ps")
                    nc.tensor.matmul(rb_ps, bcast, r_sb, start=True, stop=True)
                    rb_sb = state_pool.tile([P, W], BF16, tag="rb")
                    nc.scalar.activation(
                        out=rb_sb, in_=rb_ps,
                        func=mybir.ActivationFunctionType.Copy)
                    nc.vector.tensor_mul(
                        out=ec[:, t + 4, :], in0=ec[:, t + 4, :], in1=rb_sb)

        f_ps = psum_pool.tile([G, W], F32, tag="sps")
        nc.tensor.matmul(f_ps, astop, alpha, start=True, stop=True)
        ln_f = small_pool.tile([G, W], F32, tag="ln")
        nc.scalar.activation(
            out=ln_f, in_=f_ps, func=mybir.ActivationFunctionType.Ln,
            scale=lnscale)
        nc.vector.tensor_add(out=cacc, in0=cacc, in1=ln_f)

        nll_sb = small_pool.tile([G, J], F32, tag="nll")
        nc.vector.tensor_sub(out=nll_sb, in0=cacc[:, 0:J], in1=cacc[:, J:W])
        nc.vector.tensor_scalar_add(
            out=nll_sb, in0=nll_sb, scalar1=float(T) * (C0A - C0B))
        nc.sync.dma_start(out=out_ext[:], in_=nll_sb)

    nc.finalize()
    return nc


def _host_prep(feats, tags, transitions):
    """Build per-core input maps. Layout/dtype staging only — all FLOPs on device
    except the 11x11 exp(transitions) weight build."""
    import ml_dtypes
    f32 = np.float32
    bf16 = ml_dtypes.bfloat16
    feats = np.asarray(feats, dtype=f32)
    tags_i = np.asarray(tags).astype(np.int8)
    trans = np.asarray(transitions, dtype=f32)

    def padp(a):
        """pad partition (first) dim from PL=126 to P=128 with zeros"""
        out = np.zeros((P,) + a.shape[1:], dtype=a.dtype)
        out[:PL] = a
        return np.ascontiguousarray(out)

    A = np.exp(trans.astype(np.float64)).astype(f32)     # A[next, prev]
    Abody = A[:KT, :KT]
    eye = np.eye(G, dtype=f32)
    bd0 = np.kron(eye, Abody.T)                          # [126,126]
    bd = np.zeros((P, P), dtype=bf16)
    bd[:PL, :PL] = bd0.astype(bf16)
    astart = padp(np.tile(A[:KT, START], G)[:, None].astype(f32))
    astop = padp(np.kron(eye, A[STOP, :KT].reshape(KT, 1)).astype(bf16))
    ones_bd = padp(np.kron(eye, np.ones((KT, 1), f32)).astype(bf16))
    bcast = np.zeros((G, P), dtype=f32)
    bcast[:, :PL] = np.kron(eye, np.ones((1, KT), f32))
    kcol = padp(np.tile(np.arange(KT, dtype=np.int8), G)[:, None])
    kcol[PL:] = -1

    nslots = G * J
    in_maps = []
    for c in range(NCORES):
        fb = feats[c * BL:(c + 1) * BL, :, :KT]
        tb = tags_i[c * BL:(c + 1) * BL]
        fpad = np.zeros((nslots, T, KT), dtype=f32)
        fpad[:BL] = fb
        tpad = np.zeros((nslots, T), dtype=np.int8)
        tpad[:BL] = tb
        feats_T = padp(
            fpad.reshape(G, J, T, KT).transpose(0, 3, 2, 1).reshape(PL, T, J))
        tags_T = padp(
            np.ascontiguousarray(np.broadcast_to(
                tpad.reshape(G, J, T).transpose(0, 2, 1)[:, None, :, :],
                (G, KT, T, J))).reshape(PL, T, J))
        in_maps.append({
            "feats_t": feats_T,
            "tags_t": tags_T,
            "bd_lhst": bd,
            "astart": astart,
            "astop": astop,
            "ones_bd": ones_bd,
            "bcast": bcast,
            "kcol": kcol,
        })
    return in_maps


LAST_EXEC_NS = None


def kernel(feats, tags, transitions):
    global LAST_EXEC_NS
    in_maps = _host_prep(feats, tags, transitions)
    nc = _build_nc()
    trace = os.environ.get("KERNEL_TRACE") == "1"
    res = None
    for attempt in range(3):
        try:
            res = run_bass_kernel_spmd(
                nc, in_maps, list(range(NCORES)), trace=trace)
            break
        except Exception:
            if attempt == 2:
                raise
            # the device occasionally reports NRT_EXEC_UNIT_UNRECOVERABLE;
            # resetting the PJRT client (like a fresh process) recovers it
            import time as _time
            import jax as _jax
            try:
                _jax.clear_caches()
            except Exception:
                pass
            for fn in ("clear_backends",):
                try:
                    getattr(_jax.extend.backend, fn)()
                except Exception:
                    try:
                        getattr(_jax, fn)()
                    except Exception:
                        pass
            _time.sleep(5)
    LAST_EXEC_NS = res.exec_time_ns
    outs = []
    for c in range(NCORES):
        nll_parts = np.asarray(res.results[c]["nll"], dtype=np.float32)
        outs.append(nll_parts.reshape(-1)[:BL])
    return np.concatenate(outs).astype(np.float32)


if __name__ == "__main__":
    rng = np.random.default_rng(0)
    feats = rng.standard_normal((B, T, K), dtype=np.float32)
    tags = rng.integers(0, 9, size=(B, T), dtype=np.int64)
    trans = rng.random((K, K), dtype=np.float32)
    trans[START, :] = -10000.0
    trans[:, STOP] = -10000.0
    out = kernel(feats=feats, tags=tags, transitions=trans)
    print(out.shape, out[:4])
